# revision 2
# baseline (speedup 1.0000x reference)
"""Trainium2 Bass kernel for nn_BilinearGrounding.

Reference computation:
    encI_p[b]  = encI[b] @ K_w.T + K_b                  # [100, 768]
    logits[b]  = encT[b] @ bil_w[0] @ encI_p[b].T       # [128, 100]
                 + bil_b[0] + mask[b, 0]

Kernel strategy (v2):
  * One-time weight fold on host (deployment-style constant folding):
        M = bil_w[0] @ K_w          [768, 2048]
        cterm[b,t] = encT[b,t,:] . (bil_w[0] @ K_b)     # scalar per (b,t)
    so the device computes, per batch b:
        Y[b]      = M @ encI[b].T                       # [768, 100]
        logits[b] = encT[b] @ Y[b] + (mask[b] + bil_b + cterm[b])
  * Data-parallel over batch: 8 batches per core x 8 NeuronCores. Host
    supplies transposed, partition-chunked bf16 layouts so every matmul
    contraction dim sits on SBUF partitions; no device transposes.
  * Stage Y runs as TWO 400-column phases (batches 0-3, then 4-7).
    Each phase keeps all 6 d-chunk accumulators RESIDENT in PSUM
    (6 banks of [128,400] f32) across the whole 16-chunk contraction,
    so Y is written once per phase (12 copy ops total) instead of the
    5x spill-accumulate the v1 kernel did (64 DVE ops, 35us DVE busy).
  * Stage C (logits) interleaves into phase B: C_A (batches 0-3) runs
    after phase B's first two i-chunks (covering the phase-A spill
    latency), C_B right at the end. Phase B's last two i-chunks run
    dc-major so each accumulator's spill (alternating DVE/ACT engines)
    pipelines ahead of C_B's reads.
  * PSUM map: 6 banks acc0-5 (reused by both phases), 2 banks tag 'pc'
    rotating filler -> pcA -> pcB1 -> pcB2 (epilogue of one block reads
    its bank while the PE fills the other; stage-C groups are b-outer so
    each 100-col slice's start..stop completes before the next slice's
    start clears the bank's has_written bits).
  * DMA: both HWDGE rings stream in parallel, ordered by consumption:
      SP : mtb[0](split small first), mtb[1:9], enctA, mask | outA, outB1
      ACT: enciA chunks, mtb[9:16], enciB, enctB            | outB2
    mtb is split 9/7 across the rings because it paces phase A
    (196.6KB/chunk vs the PE's 1.01us/chunk consumption).
  * Junk bf16 fillers (on a gpsimd-memset tile, no DMA deps) keep the
    PE HAM clock busy from the end of the fixed ~8.2us framework
    preamble until the first data matmuls, so the 3.4us HAM warmup
    window completes as early as possible.
"""

import numpy as np

B, N_TOK, N_ROI = 64, 128, 100
T_HID, I_HID = 768, 2048
NCORES = 8
NB = B // NCORES          # batches per core
NCOL = NB * N_ROI         # 800  (stacked roi columns)
NTCOL = NB * N_TOK        # 1024 (stacked token columns)
IC = I_HID // 128         # 16 i-chunks (contraction for Y)
DC = T_HID // 128         # 6  d-chunks (contraction for logits)
HB = 4                    # batches per phase
HCOL = HB * N_ROI         # 400 columns per phase

FILLERS = 6
_CACHE = {}


def _build():
    import concourse.tile as tile
    from concourse import bacc, mybir
    from contextlib import ExitStack

    f32 = mybir.dt.float32
    bf16 = mybir.dt.bfloat16

    nc = bacc.Bacc("TRN2", target_bir_lowering=False)
    d_mtb = nc.dram_tensor("mtb", [I_HID, T_HID], bf16, kind="ExternalInput")
    d_enci = nc.dram_tensor("enci_t", [I_HID, NCOL], bf16, kind="ExternalInput")
    d_enct = nc.dram_tensor("enct_t", [T_HID, NTCOL], bf16, kind="ExternalInput")
    # mask (tok p, col b*100+r) with bil_b and the encT.c term folded in
    d_mask = nc.dram_tensor("maskb", [128, NCOL], f32, kind="ExternalInput")
    d_out = nc.dram_tensor("out", [NTCOL, N_ROI], f32, kind="ExternalOutput")

    mtb_r = d_mtb[:, :].rearrange("(ic p) t -> p ic t", p=128)    # [128,16,768]
    enci_r = d_enci[:, :].rearrange("(ic p) n -> p ic n", p=128)  # [128,16,800]
    enct_r = d_enct[:, :].rearrange("(dc p) n -> p dc n", p=128)  # [128,6,1024]
    out_r = d_out[:, :].rearrange("(b p) r -> p b r", p=128)      # [128,8,100]

    with tile.TileContext(nc) as tc, ExitStack() as ctx:
        sb = ctx.enter_context(tc.tile_pool(name="sb", bufs=1))
        ps = ctx.enter_context(tc.tile_pool(name="ps", bufs=1, space="PSUM"))

        MTB = sb.tile([128, IC, T_HID], bf16)     # M^T chunks (lhsT)
        ENCI = sb.tile([128, IC, NCOL], bf16)     # encI^T chunks (cols A|B)
        ENCT = sb.tile([128, DC, NTCOL], bf16)    # encT^T chunks (lhsT)
        MASK = sb.tile([128, NCOL], f32)          # mask + bil_b + encT.c
        YA = sb.tile([128, DC, HCOL], bf16)       # Y batches 0-3
        YB = sb.tile([128, DC, HCOL], bf16)       # Y batches 4-7
        OUT = sb.tile([128, NB, N_ROI], f32)
        JUNK = sb.tile([128, 512], bf16)          # filler operands (memset)

        # ---- DMA triggers, both rings, in consumption order.
        # SP ring: mtb[0:9] paces phase A chunks 0-8; first chunk split so
        # the dc=0 weights (LDW gate) land minimally early. Then enctA+mask
        # (needed ~27-28us by stage C_A).
        nc.sync.dma_start(out=MTB[:, 0, 0:128], in_=mtb_r[:, 0, 0:128])
        nc.sync.dma_start(out=MTB[:, 0, 128:T_HID], in_=mtb_r[:, 0, 128:T_HID])
        nc.sync.dma_start(out=MTB[:, 1:4, :], in_=mtb_r[:, 1:4, :])
        nc.sync.dma_start(out=MTB[:, 4:9, :], in_=mtb_r[:, 4:9, :])
        nc.sync.dma_start(out=ENCT[:, :, 0:512], in_=enct_r[:, :, 0:512])
        nc.sync.dma_start(out=MASK[:, :], in_=d_mask[:, :])
        # ACT ring: enciA chunks (light, same pacing as mtb), then mtb tail
        # (chunks 9-15 arrive ~3us ahead of the PE), enciB, enctB.
        nc.scalar.dma_start(out=ENCI[:, 0, 0:HCOL], in_=enci_r[:, 0, 0:HCOL])
        nc.scalar.dma_start(out=ENCI[:, 1:4, 0:HCOL],
                            in_=enci_r[:, 1:4, 0:HCOL])
        nc.scalar.dma_start(out=ENCI[:, 4:10, 0:HCOL],
                            in_=enci_r[:, 4:10, 0:HCOL])
        nc.scalar.dma_start(out=ENCI[:, 10:IC, 0:HCOL],
                            in_=enci_r[:, 10:IC, 0:HCOL])
        nc.scalar.dma_start(out=MTB[:, 9:IC, :], in_=mtb_r[:, 9:IC, :])
        nc.scalar.dma_start(out=ENCI[:, 0:8, HCOL:NCOL],
                            in_=enci_r[:, 0:8, HCOL:NCOL])
        nc.scalar.dma_start(out=ENCI[:, 8:IC, HCOL:NCOL],
                            in_=enci_r[:, 8:IC, HCOL:NCOL])
        nc.scalar.dma_start(out=ENCT[:, :, 512:NTCOL],
                            in_=enct_r[:, :, 512:NTCOL])

        # ---- fillers: junk bf16 matmuls with no DMA deps bridge the HAM
        # warmup from the framework preamble end (~8.2us) to the first
        # data matmul (~10.7us). One accumulation group, one PSUM bank.
        nc.gpsimd.memset(JUNK[:, :], 0.25)
        fp = ps.tile([128, 512], f32, tag="pc", bufs=2, name="fill")
        for i in range(FILLERS):
            nc.tensor.matmul(fp[:, :], JUNK[:, 0:128], JUNK[:, :],
                             start=(i == 0), stop=(i == FILLERS - 1))

        accs = [ps.tile([128, HCOL], f32, tag=f"acc{dc}", bufs=1,
                        name=f"acc{dc}") for dc in range(DC)]

        def ymm(ph, ic, dc, start, stop):
            nc.tensor.matmul(
                accs[dc][:, :], MTB[:, ic, dc * 128:(dc + 1) * 128],
                ENCI[:, ic, ph * HCOL:(ph + 1) * HCOL],
                start=start, stop=stop)

        def spill(ph, dc):
            y = YA if ph == 0 else YB
            eng = nc.vector if dc % 2 == 0 else nc.scalar
            if dc % 2 == 0:
                eng.tensor_copy(out=y[:, dc, :], in_=accs[dc][:, :])
            else:
                eng.copy(out=y[:, dc, :], in_=accs[dc][:, :])

        def stagec(pc, b0, nb, y):
            # b-outer: each 100-col slice's start..stop group completes
            # before the next slice's start clears the bank's has_written.
            for bb in range(nb):
                b = b0 + bb
                for dc in range(DC):
                    nc.tensor.matmul(
                        pc[:, bb * N_ROI:(bb + 1) * N_ROI],
                        ENCT[:, dc, b * 128:(b + 1) * 128],
                        y[:, dc, (b % HB) * N_ROI:(b % HB + 1) * N_ROI],
                        start=(dc == 0), stop=(dc == DC - 1))

        def epilogue(pc, b0, nb, store_eng):
            nc.vector.tensor_add(
                OUT[:, b0:b0 + nb, :], pc[:, :],
                MASK[:, b0 * N_ROI:(b0 + nb) * N_ROI])
            store_eng.dma_start(out=out_r[:, b0:b0 + nb, :],
                                in_=OUT[:, b0:b0 + nb, :])

        # ---- phase A: batches 0-3, ic-streamed, accs resident in PSUM
        for ic in range(IC):
            for dc in range(DC):
                ymm(0, ic, dc, start=(ic == 0), stop=(ic == IC - 1))
        for dc in range(DC):
            spill(0, dc)

        # ---- phase B chunks 0-1 first: they cover the phase-A spill
        # latency so C_A's reads of YA never stall the PE.
        for ic in range(2):
            for dc in range(DC):
                ymm(1, ic, dc, start=(ic == 0), stop=False)

        # ---- stage C for batches 0-3, epilogue + store (SP ring)
        pcA = ps.tile([128, HCOL], f32, tag="pc", bufs=2, name="pcA")
        stagec(pcA, 0, HB, YA)
        epilogue(pcA, 0, HB, nc.sync)

        # ---- phase B chunks 2-13
        for ic in range(2, IC - 2):
            for dc in range(DC):
                ymm(1, ic, dc, start=False, stop=False)
        # last two chunks dc-major: acc[dc] finishes early so its spill
        # (alternating DVE/ACT) pipelines ahead of C_B's reads.
        for dc in range(DC):
            ymm(1, IC - 2, dc, start=False, stop=False)
            ymm(1, IC - 1, dc, start=False, stop=True)
            spill(1, dc)

        # ---- stage C for batches 4-6 then 7 (separate banks so each
        # epilogue reads its bank while the PE fills the other). Last
        # store smallest and on the otherwise-idle ACT ring.
        pcB1 = ps.tile([128, 3 * N_ROI], f32, tag="pc", bufs=2, name="pcB1")
        stagec(pcB1, HB, 3, YB)
        epilogue(pcB1, HB, 3, nc.sync)
        pcB2 = ps.tile([128, N_ROI], f32, tag="pc", bufs=2, name="pcB2")
        stagec(pcB2, 7, 1, YB)
        epilogue(pcB2, 7, 1, nc.scalar)

    nc.finalize()
    return nc


def _get_nc():
    if "nc" not in _CACHE:
        _CACHE["nc"] = _build()
    return _CACHE["nc"]


def _prep_in_maps(encT, encI, mask, K_w, K_b, bil_w, bil_b):
    import ml_dtypes

    bf16 = ml_dtypes.bfloat16
    encT = np.asarray(encT, np.float32)
    encI = np.asarray(encI, np.float32)
    mask = np.asarray(mask, np.float32)
    K_w = np.asarray(K_w, np.float32)
    K_b = np.asarray(K_b, np.float32)
    bil_w = np.asarray(bil_w, np.float32)
    bil_b = np.asarray(bil_b, np.float32)

    # One-time weight fold (f64 for accuracy); folded weight ships as bf16
    M = bil_w[0].astype(np.float64) @ K_w.astype(np.float64)
    c = bil_w[0].astype(np.float64) @ K_b.astype(np.float64)
    mtb = np.ascontiguousarray(M.T).astype(bf16)                  # [2048, 768]

    in_maps = []
    for cid in range(NCORES):
        sl = slice(cid * NB, (cid + 1) * NB)
        enci_t = np.ascontiguousarray(
            encI[sl].transpose(2, 0, 1).reshape(I_HID, NCOL)).astype(bf16)
        enct_t = np.ascontiguousarray(
            encT[sl].transpose(2, 0, 1).reshape(T_HID, NTCOL)).astype(bf16)
        # cterm[b,t] = encT[b,t,:] . c — the Y bias term contracted with
        # encT on host (f64), folded into the mask epilogue tensor
        cterm = encT[sl].astype(np.float64) @ c                   # [8, 128]
        maskb = np.ascontiguousarray(
            (mask[sl, 0].transpose(1, 0, 2)                       # [128,8,100]
             + cterm.T[:, :, None]
             + np.float64(bil_b[0])).reshape(128, NCOL)).astype(np.float32)
        in_maps.append({"mtb": mtb, "enci_t": enci_t, "enct_t": enct_t,
                        "maskb": maskb})
    return in_maps


def _run(inputs: dict, trace: bool = False, tmpdir=None):
    from concourse.bass_utils import run_bass_kernel_spmd

    in_maps = _prep_in_maps(**inputs)
    nc = _get_nc()
    res = run_bass_kernel_spmd(nc, in_maps, list(range(NCORES)), trace=trace,
                               tmpdir=tmpdir)
    out = np.concatenate(
        [res.results[i]["out"].reshape(NB, N_TOK, N_ROI) for i in range(NCORES)],
        axis=0)
    return out, res


def kernel(**inputs) -> np.ndarray:
    out, _ = _run(inputs, trace=False)
    return out
